# revision 49
# baseline (speedup 1.0000x reference)
"""AttentionReadout kernel for Trainium2 (8 NeuronCores, data-parallel by chunk).

Reference computation (per full input):
    scores = (tanh(x @ W1 + b1) @ W2)[:, 0]          # [N]
    chunk_id = batch // 32                            # 32 graphs per chunk
    w = softmax of scores within each chunk           # [N]
    out = segment_sum(w[:, None] * x, batch)          # [4096, 256]

Shapes: x [262144, 256] f32, batch [262144] i64 (sorted, uniform: 64
nodes/graph), W1 [256,256], b1 [256], W2 [256,1].

Strategy (per core, 32768 nodes = 16 chunks of 2048 nodes):
  - host: ship x twice but cheaply: natural layout in bf16 (pooling rhs)
    and transposed layout in fp8-e4m3 (MLP rhs, scaled x4); W1 in fp8
    (scaled x512); the fp8 dequant scale 1/2048 is folded into the tanh
    activation's scale input.
  - device, per chunk (2048 nodes):
      hT = W1.T @ xT          (PE, fp8 DoubleRow: contraction 256 in one
                               pass, 0.5 cycles/row -> 4 matmuls/chunk)
      th = tanh(hT/2048 + b1) (ACT, psum->sbuf bf16, 4x free-1024 tiles)
      s[n] = th.T @ W2        (PE, tanh tile stationary, F=1 matmuls)
      e = exp(s), rowsum      (ACT fused accum_out)
      D = allreduce(rowsum)   (GPSIMD partition_all_reduce)
      E[n, g] = e * mask      (DVE, mask precomputed on host; the 1/D
                               softmax normalization is deferred to the
                               output scale - mathematically identical)
      outT[h,g] = x.T-contract: stationary x-tile [128n,128h], moving
                  E-tile [128n,32g] (PE, F=32: few streamed rows)
      o = outT * (1/D)        (DVE tensor_scalar, psum->sbuf)
    output is stored transposed ([128, 2, graphs]) and un-transposed on
    the host.
  - softmax max-subtraction is skipped: scores = tanh(.)@W2 are bounded by
    sum|W2| <= 16, so exp() cannot overflow in f32 and e/sum(e) is
    mathematically identical to the max-shifted form.
"""

import numpy as np
import ml_dtypes

import concourse.bass as bass
import concourse.bacc as bacc
import concourse.tile as tile
import concourse.mybir as mybir
import concourse.bass_isa as bass_isa
from concourse.bass_utils import run_bass_kernel_spmd

BF16 = mybir.dt.bfloat16
FP8 = mybir.dt.float8e4
F32 = mybir.dt.float32
NP_BF16 = ml_dtypes.bfloat16
NP_FP8 = ml_dtypes.float8_e4m3

N_CORES = 8
HIDDEN = 256
CHUNK_GRAPHS = 32
GRAPH_NODES = 64          # uniform: nodes per graph
TILE_NODES = 128          # nodes per node-tile (SBUF partition dim)
CHUNK_NODES = CHUNK_GRAPHS * GRAPH_NODES      # 2048
TILES_PER_CHUNK = CHUNK_NODES // TILE_NODES   # 16
HALF_NODES = 1024                             # tanh tile granularity

X_SCALE = 4.0             # fp8 quantization scale for x
W1_SCALE = 512.0          # fp8 quantization scale for W1
DEQ = 1.0 / (X_SCALE * W1_SCALE)   # folded into the tanh activation scale

STORE_CHUNKS = 4          # chunks per output store (512B runs per partition)

_NC_CACHE = {}


def build_nc(n_chunks, out_name="out"):
    """Build the per-core Bass program (identical across cores)."""
    nc = bacc.Bacc("TRN2", target_bir_lowering=False, debug=False,
                   enable_asserts=False)

    nodes = n_chunks * CHUNK_NODES
    n_graphs = n_chunks * CHUNK_GRAPHS
    # DRAM I/O (per-core shard)
    x_nat_d = nc.dram_tensor(
        "x_nat", [n_chunks, TILE_NODES, TILES_PER_CHUNK, HIDDEN], BF16,
        kind="ExternalInput").ap()
    x_tr8_d = nc.dram_tensor(
        "x_tr8", [2, 128, nodes], FP8, kind="ExternalInput").ap()
    w1_d = nc.dram_tensor("w1", [128, 2, 2, 128], FP8,
                          kind="ExternalInput").ap()
    w2_d = nc.dram_tensor("w2", [128, 2], BF16, kind="ExternalInput").ap()
    b1_d = nc.dram_tensor("b1", [128, 2], F32, kind="ExternalInput").ap()
    mask_d = nc.dram_tensor(
        "maskw", [TILE_NODES, CHUNK_GRAPHS, TILES_PER_CHUNK], BF16,
        kind="ExternalInput").ap()
    # output stored transposed: outT[p, hh, g] = out[g, hh*128 + p]
    out_d = nc.dram_tensor(
        out_name, [128, 2, n_graphs], F32, kind="ExternalOutput").ap()


    with tile.TileContext(nc) as tc:
        with (
            tc.tile_pool(name="consts", bufs=1) as consts,
            tc.tile_pool(name="xpool", bufs=4) as xpool,
            tc.tile_pool(name="xt8pool", bufs=4) as xt8pool,
            tc.tile_pool(name="thpool", bufs=10) as thpool,
            tc.tile_pool(name="epool", bufs=3) as epool,
            tc.tile_pool(name="opool", bufs=2) as opool,
            tc.tile_pool(name="hpsum", bufs=3, space="PSUM") as hpsum,
            tc.tile_pool(name="spsum", bufs=1, space="PSUM") as spsum,
            tc.tile_pool(name="ppsum", bufs=1, space="PSUM") as ppsum,
        ):
            # load order matters at startup: w1+b1 gate the first MLP+tanh,
            # so they go first; w2/mask (score/pool path) ride after the
            # first chunk's MLP operand.
            w1_sb = consts.tile([128, 2, 2, 128], FP8)
            nc.sync.dma_start(out=w1_sb, in_=w1_d)
            b1_sb = consts.tile([128, 2], F32)
            nc.sync.dma_start(out=b1_sb, in_=b1_d)
            w2_sb = consts.tile([128, 2], BF16)
            mask_sb = consts.tile([TILE_NODES, CHUNK_GRAPHS, TILES_PER_CHUNK],
                                  BF16)

            def emit_const_loads2():
                nc.sync.dma_start(out=w2_sb, in_=w2_d)
                nc.sync.dma_start(out=mask_sb, in_=mask_d)

            # Software pipeline, 3 chunks deep:
            #   iteration c: MLP+tanh of chunk c, score+softmax of c-1,
            #   pooling+output-scale of c-2. DMA prefetches chunk c+2.
            st = {}  # per-chunk live tiles

            def emit_load_xt8(c):
                # two half-tiles: the MLP of (mt, half) only waits for its
                # own half's DMA, halving the load latency in front of each
                # chunk's first h-matmul (matters at startup and at the tail)
                halves = []
                for half in range(2):
                    xt8_sb = xt8pool.tile([128, 2, HALF_NODES], FP8,
                                          tag=f"xt8_{half}",
                                          name=f"xt8_{c}_{half}")
                    lo = c * CHUNK_NODES + half * HALF_NODES
                    nc.sync.dma_start(
                        out=xt8_sb,
                        in_=x_tr8_d[:, :, lo:lo + HALF_NODES]
                        .transpose([1, 0, 2]))
                    halves.append(xt8_sb)
                st[c] = {"xt8": halves, "th": {}}

            def emit_load_xnat(c):
                x_sb = xpool.tile([TILE_NODES, TILES_PER_CHUNK, HIDDEN], BF16,
                                  tag="x")
                nc.sync.dma_start(out=x_sb, in_=x_nat_d[c])
                st[c]["x"] = x_sb

            def mlp_ops(c):
                """4 ops; op i = one DoubleRow matmul (contraction 256 in a
                single pass) + the tanh that consumes it. Order (mt, half):
                (0,0),(1,0),(0,1),(1,1) so score tiles 0-7 unblock first."""
                xt8_sb = st[c]["xt8"]
                s_ps = spsum.tile([128, TILES_PER_CHUNK], F32, tag="s",
                                  name=f"s_ps{c}")
                st[c]["s"] = s_ps
                ops = []
                for half in range(2):
                    for mt in range(2):
                        def op(mt=mt, half=half, c=c):
                            h_ps = hpsum.tile([128, HALF_NODES], F32, tag="h",
                                              name=f"h_ps{c}_{mt}_{half}")
                            # matmul output must fit one PSUM bank (512 f32):
                            # two 512-node blocks fill the [128, 1024] tile
                            for bb in range(2):
                                lo = bb * 512
                                nc.tensor.matmul(
                                    h_ps[:, bb * 512:(bb + 1) * 512],
                                    w1_sb[:, :, mt, :],
                                    xt8_sb[half][:, :, lo:lo + 512],
                                    start=True, stop=True,
                                    perf_mode=mybir.MatmulPerfMode.DoubleRow)
                            th = thpool.tile([128, HALF_NODES], BF16,
                                             tag="th", name=f"th{c}_{mt}_{half}")
                            nc.scalar.activation(
                                th, h_ps, mybir.ActivationFunctionType.Tanh,
                                bias=b1_sb[:, mt:mt + 1], scale=DEQ)
                            st[c]["th"][(mt, half)] = th
                        ops.append(op)
                return ops

            def emit_scores(c, half):
                """8 node-tiles of one half: accumulating F=1 matmul pairs
                with the tanh tile as the stationary operand. Emitted only
                once the tanh tiles it reads are long finished, so the
                weight loads never park in the PE wait queue."""
                s_ps = st[c]["s"]
                for tl in range(8):
                    t = half * 8 + tl
                    for mt in range(2):
                        th = st[c]["th"][(mt, half)]
                        nc.tensor.matmul(
                            s_ps[:, t:t + 1],
                            th[:, tl * 128:(tl + 1) * 128],
                            w2_sb[:, mt:mt + 1],
                            start=(mt == 0), stop=(mt == 1))

            def emit_softmax(c):
                # exp on ACT (no accum_out: the denominator row-sum runs on
                # the idle DVE instead, saving the 187ns accumulator read on
                # the bottleneck engine); allreduce on GPSIMD; E-expansion and
                # reciprocal on DVE. The 1/D normalization is deferred to the
                # output copy, so E = e * mask needs only e.
                e_sb = epool.tile([128, TILES_PER_CHUNK], BF16, tag="e")
                nc.scalar.activation(
                    e_sb, st[c]["s"], mybir.ActivationFunctionType.Exp)
                e_full = epool.tile(
                    [TILE_NODES, CHUNK_GRAPHS, TILES_PER_CHUNK], BF16,
                    tag="efull")
                e_bc = e_sb.unsqueeze(1).broadcast_to(
                    [TILE_NODES, CHUNK_GRAPHS, TILES_PER_CHUNK])
                nc.vector.tensor_mul(e_full, e_bc, mask_sb)
                st[c]["E"] = e_full
                acc = epool.tile([128, 1], F32, tag="acc")
                nc.vector.tensor_reduce(
                    acc, e_sb, axis=mybir.AxisListType.X,
                    op=mybir.AluOpType.add)
                dsum = epool.tile([128, 1], F32, tag="dsum")
                nc.gpsimd.partition_all_reduce(
                    dsum, acc, 128, bass_isa.ReduceOp.add)
                rden = epool.tile([128, 1], F32, tag="rden")
                nc.vector.reciprocal(rden, dsum)
                st[c]["rden"] = rden
                # one PSUM bank holds both h-half accumulators; the two
                # accumulation groups run sequentially (hh0 fully, then hh1),
                # which is safe in the same bank (start only zeroes the
                # region its group writes)
                p_ps = ppsum.tile([128, 2, CHUNK_GRAPHS], F32, tag="p",
                                  name=f"p_{c}")
                st[c]["p"] = p_ps

            def pool_ops(c, hh):
                """16 F=32 matmuls for one h-half: stationary x-tile
                [128n, 128h], moving E-tile [128n, 32g]."""
                ops = []
                for t in range(TILES_PER_CHUNK):
                    def op(t=t, c=c, hh=hh):
                        nc.tensor.matmul(
                            st[c]["p"][:, hh, :],
                            st[c]["x"][:, t, hh * 128:(hh + 1) * 128],
                            st[c]["E"][:, :, t],
                            start=(t == 0),
                            stop=(t == TILES_PER_CHUNK - 1),
                            skip_group_check=True)
                    ops.append(op)
                return ops

            def emit_out(c):
                """Scale pooled output by 1/D into the grouped store tile;
                DMA out every STORE_CHUNKS chunks (the final chunks flush in
                smaller pieces so the drain-path store is short)."""
                j = c % STORE_CHUNKS
                if j == 0:
                    st["ogroup"] = opool.tile(
                        [128, 2, STORE_CHUNKS, CHUNK_GRAPHS], F32, tag="o",
                        name=f"ogroup{c}")
                o_sb = st["ogroup"]
                rden = st[c]["rden"]
                nc.vector.tensor_scalar_mul(
                    o_sb[:, :, j, :], st[c]["p"], rden)
                # flush a full group normally; the final group flushes its
                # first chunks early so the very last store (on the critical
                # drain path) is one small transfer
                if c == n_chunks - 2 and j > 0:
                    c0 = c - j
                    nc.sync.dma_start(
                        out=out_d[:, :, c0 * CHUNK_GRAPHS:
                                  (c + 1) * CHUNK_GRAPHS],
                        in_=o_sb[:, :, 0:j + 1, :])
                elif c == n_chunks - 1 and j > 0:
                    nc.sync.dma_start(
                        out=out_d[:, :, c * CHUNK_GRAPHS:
                                  (c + 1) * CHUNK_GRAPHS],
                        in_=o_sb[:, :, j, :])
                elif j == STORE_CHUNKS - 1:
                    c0 = c - (STORE_CHUNKS - 1)
                    nc.sync.dma_start(
                        out=out_d[:, :, c0 * CHUNK_GRAPHS:
                                  (c + 1) * CHUNK_GRAPHS],
                        in_=o_sb)
                del st[c]

            # DMA stream ordered by need-time: xt8(c) feeds the MLP at
            # iteration c, x_nat(c) feeds pooling at iteration c+2 — so the
            # transposed operand of chunk c+2 is issued BEFORE the natural
            # operand of chunk c. This pulls the last chunk's MLP input
            # (which gates the tanh tail on the saturated ACT engine) ~2
            # transfers earlier.
            emit_load_xt8(0)
            emit_const_loads2()
            emit_load_xt8(1)
            for c in range(n_chunks + 2):
                if c + 2 < n_chunks:
                    emit_load_xt8(c + 2)
                if c < n_chunks:
                    emit_load_xnat(c)
                mops = mlp_ops(c) if c < n_chunks else None
                # PE emission order: each score half is emitted only after
                # its tanh inputs have been done for most of an iteration
                # (half 1 of chunk c-1 here, half 0 of chunk c at the very
                # end), so exp(c-1) — which gates nothing but must slot
                # between tanh instructions on the saturated ACT engine —
                # never stalls, and parked weight-loads never block the PE
                # sequencer. The trivial pool matmuls of c-2 fill the back
                # half, one accumulation group (h-half) at a time.
                if mops is not None:
                    mops[0]()
                    mops[1]()
                # pools + output of c-2 go before the score/softmax of c-1:
                # their inputs are long ready, and on the in-order PE/DVE
                # queues this keeps the drain-phase output chain from
                # queueing behind the last chunk's score round-trip
                if c >= 2:
                    for op in pool_ops(c - 2, 0):
                        op()
                    for op in pool_ops(c - 2, 1):
                        op()
                    emit_out(c - 2)
                if 1 <= c <= n_chunks:
                    # scores half-1 after BOTH leading MLP ops: their hoisted
                    # weight-loads wait on the last tanh of c-1, and placed
                    # here they cannot stall the h-matmuls feeding the next
                    # two tanh instructions
                    emit_scores(c - 1, 1)
                    emit_softmax(c - 1)
                if mops is not None:
                    mops[2]()
                    mops[3]()
                if c < n_chunks:
                    emit_scores(c, 0)

    nc.compile()
    return nc


def _prep_inputs(x, W1, b1, W2, n_chunks_per_core):
    """Host-side marshalling: dtype casts, layouts, masks. Returns in_maps."""
    N, H = x.shape
    nodes_per_core = n_chunks_per_core * CHUNK_NODES

    xf = np.asarray(x, dtype=np.float32)
    xb = xf.astype(NP_BF16)

    # natural layout: [core, chunk, p, t, h] bf16
    x_nat = np.ascontiguousarray(
        xb.reshape(N_CORES, n_chunks_per_core, TILES_PER_CHUNK, TILE_NODES, H)
        .transpose(0, 1, 3, 2, 4))
    # transposed layout: [core, kt, q, n_local] fp8, scaled by X_SCALE
    x_tr8 = np.ascontiguousarray(
        (xf * X_SCALE).astype(NP_FP8)
        .reshape(N_CORES, nodes_per_core, H).transpose(0, 2, 1)
        .reshape(N_CORES, 2, 128, nodes_per_core))

    W1f = np.asarray(W1, dtype=np.float32)
    w1_host = np.ascontiguousarray(
        (W1f * W1_SCALE).astype(NP_FP8)
        .reshape(2, 128, 2, 128).transpose(1, 0, 2, 3))  # [p, kt, mt, j]
    w2_host = np.ascontiguousarray(
        np.asarray(W2).astype(NP_BF16).reshape(2, 128).T)   # [p, mt]
    b1_host = np.ascontiguousarray(
        np.asarray(b1).astype(np.float32).reshape(2, 128).T)  # [p, mt]

    # mask[p, g, t] = 1 iff node (t, p) of a chunk belongs to graph g
    p_idx = np.arange(TILE_NODES)
    t_idx = np.arange(TILES_PER_CHUNK)
    g_of_pt = 2 * t_idx[None, :] + p_idx[:, None] // GRAPH_NODES  # [p, t]
    mask_host = (g_of_pt[:, None, :] ==
                 np.arange(CHUNK_GRAPHS)[None, :, None]).astype(NP_BF16)

    in_maps = []
    for core in range(N_CORES):
        in_maps.append({
            "x_nat": x_nat[core],
            "x_tr8": x_tr8[core],
            "w1": w1_host,
            "w2": w2_host,
            "b1": b1_host,
            "maskw": mask_host,
        })
    return in_maps


def _reference_numpy(x, batch, W1, b1, W2):
    """Fallback for non-uniform batch layouts: straight numpy."""
    x = np.asarray(x, dtype=np.float64)
    batch = np.asarray(batch).astype(np.int64)
    # the reference uses a fixed segment count (num_graphs = num_nodes/64),
    # not batch.max()+1 — keep trailing empty graphs as zero rows
    n_graphs = max(int(batch.max()) + 1, x.shape[0] // GRAPH_NODES)
    scores = np.tanh(x @ np.asarray(W1, np.float64) +
                     np.asarray(b1, np.float64)) @ np.asarray(W2, np.float64)
    scores = scores[:, 0]
    chunk_id = batch // CHUNK_GRAPHS
    n_chunks = int(chunk_id.max()) + 1
    m = np.full(n_chunks, -np.inf)
    np.maximum.at(m, chunk_id, scores)
    e = np.exp(scores - m[chunk_id])
    denom = np.zeros(n_chunks)
    np.add.at(denom, chunk_id, e)
    w = e / denom[chunk_id]
    out = np.zeros((n_graphs, x.shape[1]))
    np.add.at(out, batch, w[:, None] * x)
    return out.astype(np.float32)


def kernel(x, batch, W1, b1, W2, trace=False):
    x = np.asarray(x)
    batch = np.asarray(batch)
    N, H = x.shape
    n_graphs = int(batch[-1]) + 1

    # This kernel is specialized for the uniform sorted batch that the
    # reference generator produces (64 nodes per graph). Anything else
    # falls back to a host computation.
    expected = (np.arange(N, dtype=np.int64) * n_graphs) // N
    if (H != HIDDEN or N % (N_CORES * CHUNK_NODES) != 0
            or n_graphs % (N_CORES * CHUNK_GRAPHS) != 0
            or not np.array_equal(batch.astype(np.int64), expected)):
        return _reference_numpy(x, batch, W1, b1, W2)

    n_chunks_per_core = N // (N_CORES * CHUNK_NODES)

    if n_chunks_per_core not in _NC_CACHE:
        _NC_CACHE[n_chunks_per_core] = build_nc(n_chunks_per_core)
    nc = _NC_CACHE[n_chunks_per_core]

    in_maps = _prep_inputs(x, W1, b1, W2, n_chunks_per_core)
    try:
        res = run_bass_kernel_spmd(nc, in_maps, core_ids=list(range(N_CORES)),
                                   trace=trace)
    except ModuleNotFoundError:
        # NTFF trace hooks unavailable in this environment
        res = run_bass_kernel_spmd(nc, in_maps, core_ids=list(range(N_CORES)),
                                   trace=False)
    # un-transpose: outT [128, 2, graphs] -> out [graphs, 256]
    outs = []
    for r in res.results:
        ot = np.asarray(r["out"])  # [128, 2, graphs_per_core]
        outs.append(ot.transpose(2, 1, 0).reshape(-1, HIDDEN))
    out = np.concatenate(outs, axis=0)
    if trace:
        kernel.last_results = res
    return out.astype(np.float32)


# revision 64
# speedup vs baseline: 1.0021x; 1.0021x over previous
"""AttentionReadout kernel for Trainium2 (8 NeuronCores, data-parallel by chunk).

Reference computation (per full input):
    scores = (tanh(x @ W1 + b1) @ W2)[:, 0]          # [N]
    chunk_id = batch // 32                            # 32 graphs per chunk
    w = softmax of scores within each chunk           # [N]
    out = segment_sum(w[:, None] * x, batch)          # [4096, 256]

Shapes: x [262144, 256] f32, batch [262144] i64 (sorted, uniform: 64
nodes/graph), W1 [256,256], b1 [256], W2 [256,1].

Strategy (per core, 32768 nodes = 16 chunks of 2048 nodes):
  - host: ship x twice but cheaply: natural layout in bf16 (pooling rhs)
    and transposed layout in fp8-e4m3 (MLP rhs, scaled x4); W1 in fp8
    (scaled x512); the fp8 dequant scale 1/2048 is folded into the tanh
    activation's scale input.
  - device, per chunk (2048 nodes):
      hT = W1.T @ xT          (PE, fp8 DoubleRow: contraction 256 in one
                               pass, 0.5 cycles/row -> 4 matmuls/chunk)
      th = tanh(hT/2048 + b1) (ACT, psum->sbuf bf16, 4x free-1024 tiles)
      s[n] = th.T @ W2        (PE, tanh tile stationary, F=1 matmuls)
      e = exp(s), rowsum      (ACT fused accum_out)
      D = allreduce(rowsum)   (GPSIMD partition_all_reduce)
      E[n, g] = e * mask      (DVE, mask precomputed on host; the 1/D
                               softmax normalization is deferred to the
                               output scale - mathematically identical)
      outT[h,g] = x.T-contract: stationary x-tile [128n,128h], moving
                  E-tile [128n,32g] (PE, F=32: few streamed rows)
      o = outT * (1/D)        (DVE tensor_scalar, psum->sbuf)
    output is stored transposed ([128, 2, graphs]) and un-transposed on
    the host.
  - softmax max-subtraction is skipped: scores = tanh(.)@W2 are bounded by
    sum|W2| <= 16, so exp() cannot overflow in f32 and e/sum(e) is
    mathematically identical to the max-shifted form.
"""

import numpy as np
import ml_dtypes

import concourse.bass as bass
import concourse.bacc as bacc
import concourse.tile as tile
import concourse.mybir as mybir
import concourse.bass_isa as bass_isa
from concourse.bass_utils import run_bass_kernel_spmd

BF16 = mybir.dt.bfloat16
FP8 = mybir.dt.float8e4
F32 = mybir.dt.float32
NP_BF16 = ml_dtypes.bfloat16
NP_FP8 = ml_dtypes.float8_e4m3

N_CORES = 8
HIDDEN = 256
CHUNK_GRAPHS = 32
GRAPH_NODES = 64          # uniform: nodes per graph
TILE_NODES = 128          # nodes per node-tile (SBUF partition dim)
CHUNK_NODES = CHUNK_GRAPHS * GRAPH_NODES      # 2048
TILES_PER_CHUNK = CHUNK_NODES // TILE_NODES   # 16
HALF_NODES = 1024                             # tanh tile granularity

X_SCALE = 4.0             # fp8 quantization scale for x
W1_SCALE = 512.0          # fp8 quantization scale for W1
DEQ = 1.0 / (X_SCALE * W1_SCALE)   # folded into the tanh activation scale

STORE_CHUNKS = 8          # chunks per output store; bf16 output staging
                          # gives 512B runs per partition at this grouping
                          # (the host casts back to f32; quantization adds
                          # ~0.2% rms / 0.4% worst-case, well inside budget)

_NC_CACHE = {}


def build_nc(n_chunks, out_name="out"):
    """Build the per-core Bass program (identical across cores)."""
    nc = bacc.Bacc("TRN2", target_bir_lowering=False, debug=False,
                   enable_asserts=False)

    nodes = n_chunks * CHUNK_NODES
    n_graphs = n_chunks * CHUNK_GRAPHS
    # DRAM I/O (per-core shard)
    x_nat_d = nc.dram_tensor(
        "x_nat", [n_chunks, TILE_NODES, TILES_PER_CHUNK, HIDDEN], BF16,
        kind="ExternalInput").ap()
    x_tr8_d = nc.dram_tensor(
        "x_tr8", [2, 128, nodes], FP8, kind="ExternalInput").ap()
    w1_d = nc.dram_tensor("w1", [128, 2, 2, 128], FP8,
                          kind="ExternalInput").ap()
    w2_d = nc.dram_tensor("w2", [128, 2], BF16, kind="ExternalInput").ap()
    b1_d = nc.dram_tensor("b1", [128, 2], F32, kind="ExternalInput").ap()
    mask_d = nc.dram_tensor(
        "maskw", [TILE_NODES, CHUNK_GRAPHS, TILES_PER_CHUNK], BF16,
        kind="ExternalInput").ap()
    # output stored transposed: outT[p, hh, g] = out[g, hh*128 + p]
    out_d = nc.dram_tensor(
        out_name, [128, 2, n_graphs], BF16, kind="ExternalOutput").ap()


    with tile.TileContext(nc) as tc:
        with (
            tc.tile_pool(name="consts", bufs=1) as consts,
            tc.tile_pool(name="xpool", bufs=4) as xpool,
            tc.tile_pool(name="xt8pool", bufs=4) as xt8pool,
            tc.tile_pool(name="thpool", bufs=10) as thpool,
            tc.tile_pool(name="epool", bufs=3) as epool,
            tc.tile_pool(name="opool", bufs=2) as opool,
            tc.tile_pool(name="hpsum", bufs=3, space="PSUM") as hpsum,
            tc.tile_pool(name="spsum", bufs=1, space="PSUM") as spsum,
            tc.tile_pool(name="ppsum", bufs=1, space="PSUM") as ppsum,
        ):
            # startup load order: the first chunk's first xt8 half is the
            # longest transfer gating the first tanh, so it goes first; the
            # tiny w1/b1 (also gating it) ride just behind; w2/mask later.
            w1_sb = consts.tile([128, 2, 2, 128], FP8)
            b1_sb = consts.tile([128, 2], F32)
            w2_sb = consts.tile([128, 2], BF16)
            mask_sb = consts.tile([TILE_NODES, CHUNK_GRAPHS, TILES_PER_CHUNK],
                                  BF16)

            def emit_const_loads1():
                nc.sync.dma_start(out=w1_sb, in_=w1_d)
                nc.sync.dma_start(out=b1_sb, in_=b1_d)

            def emit_const_loads2():
                nc.sync.dma_start(out=w2_sb, in_=w2_d)
                nc.sync.dma_start(out=mask_sb, in_=mask_d)

            # Software pipeline, 3 chunks deep:
            #   iteration c: MLP+tanh of chunk c, score+softmax of c-1,
            #   pooling+output-scale of c-2. DMA prefetches chunk c+2.
            st = {}  # per-chunk live tiles

            def emit_load_xt8(c, between=None):
                # two half-tiles: the MLP of (mt, half) only waits for its
                # own half's DMA, halving the load latency in front of each
                # chunk's first h-matmul (matters at startup and at the tail)
                halves = []
                for half in range(2):
                    if half == 1 and between is not None:
                        between()
                    xt8_sb = xt8pool.tile([128, 2, HALF_NODES], FP8,
                                          tag=f"xt8_{half}",
                                          name=f"xt8_{c}_{half}")
                    lo = c * CHUNK_NODES + half * HALF_NODES
                    nc.sync.dma_start(
                        out=xt8_sb,
                        in_=x_tr8_d[:, :, lo:lo + HALF_NODES]
                        .transpose([1, 0, 2]))
                    halves.append(xt8_sb)
                st[c] = {"xt8": halves, "th": {}}

            def emit_load_xnat(c):
                x_sb = xpool.tile([TILE_NODES, TILES_PER_CHUNK, HIDDEN], BF16,
                                  tag="x")
                nc.sync.dma_start(out=x_sb, in_=x_nat_d[c])
                st[c]["x"] = x_sb

            def mlp_ops(c):
                """4 ops; op i = one DoubleRow matmul (contraction 256 in a
                single pass) + the tanh that consumes it. Order (mt, half):
                (0,0),(1,0),(0,1),(1,1) so score tiles 0-7 unblock first."""
                xt8_sb = st[c]["xt8"]
                s_ps = spsum.tile([128, TILES_PER_CHUNK], F32, tag="s",
                                  name=f"s_ps{c}")
                st[c]["s"] = s_ps
                ops = []
                for half in range(2):
                    for mt in range(2):
                        def op(mt=mt, half=half, c=c):
                            h_ps = hpsum.tile([128, HALF_NODES], F32, tag="h",
                                              name=f"h_ps{c}_{mt}_{half}")
                            # matmul output must fit one PSUM bank (512 f32):
                            # two 512-node blocks fill the [128, 1024] tile
                            for bb in range(2):
                                lo = bb * 512
                                nc.tensor.matmul(
                                    h_ps[:, bb * 512:(bb + 1) * 512],
                                    w1_sb[:, :, mt, :],
                                    xt8_sb[half][:, :, lo:lo + 512],
                                    start=True, stop=True,
                                    perf_mode=mybir.MatmulPerfMode.DoubleRow)
                            th = thpool.tile([128, HALF_NODES], BF16,
                                             tag="th", name=f"th{c}_{mt}_{half}")
                            nc.scalar.activation(
                                th, h_ps, mybir.ActivationFunctionType.Tanh,
                                bias=b1_sb[:, mt:mt + 1], scale=DEQ)
                            st[c]["th"][(mt, half)] = th
                        ops.append(op)
                return ops

            def emit_scores(c, half):
                """8 node-tiles of one half: accumulating F=1 matmul pairs
                with the tanh tile as the stationary operand. Emitted only
                once the tanh tiles it reads are long finished, so the
                weight loads never park in the PE wait queue."""
                s_ps = st[c]["s"]
                for tl in range(8):
                    t = half * 8 + tl
                    for mt in range(2):
                        th = st[c]["th"][(mt, half)]
                        nc.tensor.matmul(
                            s_ps[:, t:t + 1],
                            th[:, tl * 128:(tl + 1) * 128],
                            w2_sb[:, mt:mt + 1],
                            start=(mt == 0), stop=(mt == 1))

            def emit_softmax(c):
                # exp on ACT (no accum_out: the denominator row-sum runs on
                # the idle DVE instead, saving the 187ns accumulator read on
                # the bottleneck engine); allreduce on GPSIMD; E-expansion and
                # reciprocal on DVE. The 1/D normalization is deferred to the
                # output copy, so E = e * mask needs only e.
                e_sb = epool.tile([128, TILES_PER_CHUNK], BF16, tag="e")
                nc.scalar.activation(
                    e_sb, st[c]["s"], mybir.ActivationFunctionType.Exp)
                e_full = epool.tile(
                    [TILE_NODES, CHUNK_GRAPHS, TILES_PER_CHUNK], BF16,
                    tag="efull")
                e_bc = e_sb.unsqueeze(1).broadcast_to(
                    [TILE_NODES, CHUNK_GRAPHS, TILES_PER_CHUNK])
                nc.vector.tensor_mul(e_full, e_bc, mask_sb)
                st[c]["E"] = e_full
                acc = epool.tile([128, 1], F32, tag="acc")
                nc.vector.tensor_reduce(
                    acc, e_sb, axis=mybir.AxisListType.X,
                    op=mybir.AluOpType.add)
                dsum = epool.tile([128, 1], F32, tag="dsum")
                nc.gpsimd.partition_all_reduce(
                    dsum, acc, 128, bass_isa.ReduceOp.add)
                rden = epool.tile([128, 1], F32, tag="rden")
                nc.vector.reciprocal(rden, dsum)
                st[c]["rden"] = rden
                # one PSUM bank holds both h-half accumulators; the two
                # accumulation groups run sequentially (hh0 fully, then hh1),
                # which is safe in the same bank (start only zeroes the
                # region its group writes)
                p_ps = ppsum.tile([128, 2, CHUNK_GRAPHS], F32, tag="p",
                                  name=f"p_{c}")
                st[c]["p"] = p_ps

            def pool_ops(c, hh):
                """16 F=32 matmuls for one h-half: stationary x-tile
                [128n, 128h], moving E-tile [128n, 32g]."""
                ops = []
                for t in range(TILES_PER_CHUNK):
                    def op(t=t, c=c, hh=hh):
                        nc.tensor.matmul(
                            st[c]["p"][:, hh, :],
                            st[c]["x"][:, t, hh * 128:(hh + 1) * 128],
                            st[c]["E"][:, :, t],
                            start=(t == 0),
                            stop=(t == TILES_PER_CHUNK - 1),
                            skip_group_check=True)
                    ops.append(op)
                return ops

            def emit_out(c):
                """Scale pooled output by 1/D into the grouped store tile;
                DMA out every STORE_CHUNKS chunks (the final chunks flush in
                smaller pieces so the drain-path store is short)."""
                j = c % STORE_CHUNKS
                if j == 0:
                    st["ogroup"] = opool.tile(
                        [128, 2, STORE_CHUNKS, CHUNK_GRAPHS], BF16, tag="o",
                        name=f"ogroup{c}")
                o_sb = st["ogroup"]
                rden = st[c]["rden"]
                nc.vector.tensor_scalar_mul(
                    o_sb[:, :, j, :], st[c]["p"], rden)
                # flush a full group normally; the final group flushes its
                # first chunks early so the very last store (on the critical
                # drain path) is one small transfer
                if c == n_chunks - 2 and j > 0:
                    c0 = c - j
                    nc.sync.dma_start(
                        out=out_d[:, :, c0 * CHUNK_GRAPHS:
                                  (c + 1) * CHUNK_GRAPHS],
                        in_=o_sb[:, :, 0:j + 1, :])
                elif c == n_chunks - 1 and j > 0:
                    nc.sync.dma_start(
                        out=out_d[:, :, c * CHUNK_GRAPHS:
                                  (c + 1) * CHUNK_GRAPHS],
                        in_=o_sb[:, :, j, :])
                elif j == STORE_CHUNKS - 1:
                    c0 = c - (STORE_CHUNKS - 1)
                    nc.sync.dma_start(
                        out=out_d[:, :, c0 * CHUNK_GRAPHS:
                                  (c + 1) * CHUNK_GRAPHS],
                        in_=o_sb)
                del st[c]

            # DMA stream ordered by need-time: xt8(c) feeds the MLP at
            # iteration c, x_nat(c) feeds pooling at iteration c+2 — so the
            # transposed operand of chunk c+2 is issued BEFORE the natural
            # operand of chunk c. This pulls the last chunk's MLP input
            # (which gates the tanh tail on the saturated ACT engine) ~2
            # transfers earlier.
            emit_const_loads1()
            emit_load_xt8(0)
            emit_const_loads2()
            emit_load_xt8(1)
            for c in range(n_chunks + 2):
                if c + 2 < n_chunks:
                    emit_load_xt8(c + 2)
                if c < n_chunks:
                    emit_load_xnat(c)
                mops = mlp_ops(c) if c < n_chunks else None
                # PE emission order: each score half is emitted only after
                # its tanh inputs have been done for most of an iteration
                # (half 1 of chunk c-1 here, half 0 of chunk c at the very
                # end), so exp(c-1) — which gates nothing but must slot
                # between tanh instructions on the saturated ACT engine —
                # never stalls, and parked weight-loads never block the PE
                # sequencer. The trivial pool matmuls of c-2 fill the back
                # half, one accumulation group (h-half) at a time.
                if mops is not None:
                    mops[0]()
                    mops[1]()
                # pools + output of c-2 go before the score/softmax of c-1:
                # their inputs are long ready, and on the in-order PE/DVE
                # queues this keeps the drain-phase output chain from
                # queueing behind the last chunk's score round-trip
                if c >= 2:
                    for op in pool_ops(c - 2, 0):
                        op()
                    for op in pool_ops(c - 2, 1):
                        op()
                    emit_out(c - 2)
                if 1 <= c <= n_chunks:
                    # scores half-1 after BOTH leading MLP ops: their hoisted
                    # weight-loads wait on the last tanh of c-1, and placed
                    # here they cannot stall the h-matmuls feeding the next
                    # two tanh instructions
                    emit_scores(c - 1, 1)
                    emit_softmax(c - 1)
                if mops is not None:
                    mops[2]()
                    mops[3]()
                if c < n_chunks:
                    emit_scores(c, 0)

    nc.compile()
    return nc


def _prep_inputs(x, W1, b1, W2, n_chunks_per_core):
    """Host-side marshalling: dtype casts, layouts, masks. Returns in_maps."""
    N, H = x.shape
    nodes_per_core = n_chunks_per_core * CHUNK_NODES

    xf = np.asarray(x, dtype=np.float32)
    xb = xf.astype(NP_BF16)

    # natural layout: [core, chunk, p, t, h] bf16
    x_nat = np.ascontiguousarray(
        xb.reshape(N_CORES, n_chunks_per_core, TILES_PER_CHUNK, TILE_NODES, H)
        .transpose(0, 1, 3, 2, 4))
    # transposed layout: [core, kt, q, n_local] fp8, scaled by X_SCALE
    x_tr8 = np.ascontiguousarray(
        (xf * X_SCALE).astype(NP_FP8)
        .reshape(N_CORES, nodes_per_core, H).transpose(0, 2, 1)
        .reshape(N_CORES, 2, 128, nodes_per_core))

    W1f = np.asarray(W1, dtype=np.float32)
    w1_host = np.ascontiguousarray(
        (W1f * W1_SCALE).astype(NP_FP8)
        .reshape(2, 128, 2, 128).transpose(1, 0, 2, 3))  # [p, kt, mt, j]
    w2_host = np.ascontiguousarray(
        np.asarray(W2).astype(NP_BF16).reshape(2, 128).T)   # [p, mt]
    b1_host = np.ascontiguousarray(
        np.asarray(b1).astype(np.float32).reshape(2, 128).T)  # [p, mt]

    # mask[p, g, t] = 1 iff node (t, p) of a chunk belongs to graph g
    p_idx = np.arange(TILE_NODES)
    t_idx = np.arange(TILES_PER_CHUNK)
    g_of_pt = 2 * t_idx[None, :] + p_idx[:, None] // GRAPH_NODES  # [p, t]
    mask_host = (g_of_pt[:, None, :] ==
                 np.arange(CHUNK_GRAPHS)[None, :, None]).astype(NP_BF16)

    in_maps = []
    for core in range(N_CORES):
        in_maps.append({
            "x_nat": x_nat[core],
            "x_tr8": x_tr8[core],
            "w1": w1_host,
            "w2": w2_host,
            "b1": b1_host,
            "maskw": mask_host,
        })
    return in_maps


def _reference_numpy(x, batch, W1, b1, W2):
    """Fallback for non-uniform batch layouts: straight numpy."""
    x = np.asarray(x, dtype=np.float64)
    batch = np.asarray(batch).astype(np.int64)
    # the reference uses a fixed segment count (num_graphs = num_nodes/64),
    # not batch.max()+1 — keep trailing empty graphs as zero rows
    n_graphs = max(int(batch.max()) + 1, x.shape[0] // GRAPH_NODES)
    scores = np.tanh(x @ np.asarray(W1, np.float64) +
                     np.asarray(b1, np.float64)) @ np.asarray(W2, np.float64)
    scores = scores[:, 0]
    chunk_id = batch // CHUNK_GRAPHS
    n_chunks = int(chunk_id.max()) + 1
    m = np.full(n_chunks, -np.inf)
    np.maximum.at(m, chunk_id, scores)
    e = np.exp(scores - m[chunk_id])
    denom = np.zeros(n_chunks)
    np.add.at(denom, chunk_id, e)
    w = e / denom[chunk_id]
    out = np.zeros((n_graphs, x.shape[1]))
    np.add.at(out, batch, w[:, None] * x)
    return out.astype(np.float32)


def kernel(x, batch, W1, b1, W2, trace=False):
    x = np.asarray(x)
    batch = np.asarray(batch)
    N, H = x.shape
    n_graphs = int(batch[-1]) + 1

    # This kernel is specialized for the uniform sorted batch that the
    # reference generator produces (64 nodes per graph). Anything else
    # falls back to a host computation.
    expected = (np.arange(N, dtype=np.int64) * n_graphs) // N
    if (H != HIDDEN or N % (N_CORES * CHUNK_NODES) != 0
            or n_graphs % (N_CORES * CHUNK_GRAPHS) != 0
            or not np.array_equal(batch.astype(np.int64), expected)):
        return _reference_numpy(x, batch, W1, b1, W2)

    n_chunks_per_core = N // (N_CORES * CHUNK_NODES)

    if n_chunks_per_core not in _NC_CACHE:
        _NC_CACHE[n_chunks_per_core] = build_nc(n_chunks_per_core)
    nc = _NC_CACHE[n_chunks_per_core]

    in_maps = _prep_inputs(x, W1, b1, W2, n_chunks_per_core)
    try:
        res = run_bass_kernel_spmd(nc, in_maps, core_ids=list(range(N_CORES)),
                                   trace=trace)
    except ModuleNotFoundError:
        # NTFF trace hooks unavailable in this environment
        res = run_bass_kernel_spmd(nc, in_maps, core_ids=list(range(N_CORES)),
                                   trace=False)
    # un-transpose: outT [128, 2, graphs] -> out [graphs, 256]
    outs = []
    for r in res.results:
        ot = np.asarray(r["out"]).astype(np.float32)  # [128, 2, g/core]
        outs.append(ot.transpose(2, 1, 0).reshape(-1, HIDDEN))
    out = np.concatenate(outs, axis=0)
    if trace:
        kernel.last_results = res
    return out.astype(np.float32)
